# revision 4
# baseline (speedup 1.0000x reference)
"""Elementwise add (out = inp + noise) on 8 TRN2 NeuronCores.

Full inputs are (4096, 8192) fp32; batch dim is sharded 8 ways -> each core
streams 512x8192 per tensor: load inp tile, load noise tile, DVE add, store.
Memory-bound; tiles sized >=1 MiB per DMA for near-peak HBM bandwidth.
"""

import numpy as np

import concourse.tile as tile
from concourse import bacc, mybir
from concourse.bass_utils import run_bass_kernel_spmd

BATCH = 4096
FEAT = 8192
NCORES = 8
ROWS = BATCH // NCORES  # 512 rows per core
P = 128  # SBUF partitions

# Tunables
CHUNK_COLS = 4096  # columns per tile -> 128*4096*4B = 2 MiB per DMA
BUFS = 3

_nc_cache = {}


def _build_nc(chunk_cols=CHUNK_COLS, bufs=BUFS):
    key = (chunk_cols, bufs)
    if key in _nc_cache:
        return _nc_cache[key]

    # Bacc (not bass.Bass): its finalize() runs the pass pipeline incl.
    # generate_event_semaphores, which splits multi-sem waits — TRN2 allows
    # at most 1 embedded wait per instruction and walrus rejects more.
    nc = bacc.Bacc("TRN2", target_bir_lowering=False)
    f32 = mybir.dt.float32
    inp = nc.dram_tensor("inp", [ROWS, FEAT], f32, kind="ExternalInput")
    noise = nc.dram_tensor("noise", [ROWS, FEAT], f32, kind="ExternalInput")
    out = nc.dram_tensor("out", [ROWS, FEAT], f32, kind="ExternalOutput")

    n_row_tiles = ROWS // P
    n_col_tiles = FEAT // chunk_cols

    with tile.TileContext(nc) as tc:
        with tc.tile_pool(name="io", bufs=bufs) as pool:
            for i in range(n_row_tiles):
                r = slice(i * P, (i + 1) * P)
                for j in range(n_col_tiles):
                    c = slice(j * chunk_cols, (j + 1) * chunk_cols)
                    a = pool.tile([P, chunk_cols], f32, tag="a")
                    nc.sync.dma_start(a[:], inp[r, c])
                    b = pool.tile([P, chunk_cols], f32, tag="b")
                    nc.sync.dma_start(b[:], noise[r, c])
                    nc.vector.tensor_add(a[:], a[:], b[:])
                    nc.sync.dma_start(out[r, c], a[:])

    nc.finalize()
    _nc_cache[key] = nc
    return nc


def _run(inp, noise, trace=False, **spmd_kwargs):
    nc = _build_nc()
    inp = np.ascontiguousarray(inp, dtype=np.float32)
    noise = np.ascontiguousarray(noise, dtype=np.float32)
    in_maps = [
        {
            "inp": inp[i * ROWS : (i + 1) * ROWS],
            "noise": noise[i * ROWS : (i + 1) * ROWS],
        }
        for i in range(NCORES)
    ]
    res = run_bass_kernel_spmd(
        nc, in_maps, core_ids=list(range(NCORES)), trace=trace, **spmd_kwargs
    )
    full = np.concatenate([r["out"] for r in res.results], axis=0)
    return full, res


def kernel(inp, noise):
    out, _ = _run(inp, noise, trace=False)
    return out


# revision 13
# speedup vs baseline: 1.1477x; 1.1477x over previous
"""Elementwise add (out = inp + noise) on 8 TRN2 NeuronCores.

Full inputs are (4096, 8192) fp32; batch dim is sharded 8 ways -> each core
streams 512x8192 per tensor: load inp tile, load noise tile, DVE add, store.
Memory-bound; tiles sized >=1 MiB per DMA for near-peak HBM bandwidth.
"""

import numpy as np

import concourse.tile as tile
from concourse import bacc, mybir
from concourse.bass_utils import run_bass_kernel_spmd

BATCH = 4096
FEAT = 8192
NCORES = 8
ROWS = BATCH // NCORES  # 512 rows per core
P = 128  # SBUF partitions

# Tunables (picked by on-device sweep: ~132-154 us, vs ~175 us baseline)
CHUNK_COLS = 4096  # columns per tile -> 128*4096*4B = 2 MiB per DMA
BUFS = 4
LOAD_ENGS = ("sync|scalar", "scalar|sync")  # alternate HWDGE queues per iter
STORE_ENG = "sync|scalar"

_nc_cache = {}


def _build_nc(
    chunk_cols=CHUNK_COLS,
    bufs=BUFS,
    load_engs=LOAD_ENGS,
    store_eng=STORE_ENG,
    add_engs=("vector",),
):
    key = (chunk_cols, bufs, load_engs, store_eng, add_engs)
    if key in _nc_cache:
        return _nc_cache[key]

    # Bacc (not bass.Bass): its finalize() runs the pass pipeline incl.
    # generate_event_semaphores, which splits multi-sem waits — TRN2 allows
    # at most 1 embedded wait per instruction and walrus rejects more.
    nc = bacc.Bacc("TRN2", target_bir_lowering=False)
    f32 = mybir.dt.float32
    inp = nc.dram_tensor("inp", [ROWS, FEAT], f32, kind="ExternalInput")
    noise = nc.dram_tensor("noise", [ROWS, FEAT], f32, kind="ExternalInput")
    out = nc.dram_tensor("out", [ROWS, FEAT], f32, kind="ExternalOutput")

    n_row_tiles = ROWS // P
    n_col_tiles = FEAT // chunk_cols

    l0p = load_engs[0].split("|")
    l1p = load_engs[1].split("|")
    sep = store_eng.split("|")

    it = 0
    with tile.TileContext(nc) as tc:
        with tc.tile_pool(name="io", bufs=bufs) as pool:
            for i in range(n_row_tiles):
                r = slice(i * P, (i + 1) * P)
                for j in range(n_col_tiles):
                    c = slice(j * chunk_cols, (j + 1) * chunk_cols)
                    a = pool.tile([P, chunk_cols], f32, tag="a")
                    getattr(nc, l0p[it % len(l0p)]).dma_start(a[:], inp[r, c])
                    b = pool.tile([P, chunk_cols], f32, tag="b")
                    getattr(nc, l1p[it % len(l1p)]).dma_start(b[:], noise[r, c])
                    ae = add_engs[it % len(add_engs)]
                    if ae == "scalar":
                        nc.scalar.add(a[:], a[:], b[:])
                    else:
                        getattr(nc, ae).tensor_add(a[:], a[:], b[:])
                    getattr(nc, sep[it % len(sep)]).dma_start(out[r, c], a[:])
                    it += 1

    nc.finalize()
    _nc_cache[key] = nc
    return nc


def _run(inp, noise, trace=False, **spmd_kwargs):
    nc = _build_nc()
    inp = np.ascontiguousarray(inp, dtype=np.float32)
    noise = np.ascontiguousarray(noise, dtype=np.float32)
    in_maps = [
        {
            "inp": inp[i * ROWS : (i + 1) * ROWS],
            "noise": noise[i * ROWS : (i + 1) * ROWS],
        }
        for i in range(NCORES)
    ]
    res = run_bass_kernel_spmd(
        nc, in_maps, core_ids=list(range(NCORES)), trace=trace, **spmd_kwargs
    )
    full = np.concatenate([r["out"] for r in res.results], axis=0)
    return full, res


def kernel(inp, noise):
    out, _ = _run(inp, noise, trace=False)
    return out
